# revision 1
# baseline (speedup 1.0000x reference)
"""Multi-head attention (QKV proj + RoPE + softmax attention + o-proj) on 8
Trainium2 NeuronCores.

Sharding: data-parallel over batch (B=2) x tensor-parallel over heads
(16 heads -> 4 groups of 4). Core c handles batch c//4, heads 4*(c%4)..+4.
qkv_proj is column-parallel, o_proj row-parallel; each core returns a
partial o-proj output (fp16) and the host sums the 4 partials per batch.

All matmuls run in fp16 (full PE speed) with fp32 PSUM accumulation.

Schedule notes (per core):
 - DMA dispatch is the head bottleneck (~0.7us of sync-engine time per
   dma_start), so loads are batched into few large dispatches, split
   across the two HW-DGE queues (sync + activation), ordered so the first
   q-projection chain starts ~10us in and never starves after.
 - q,k are produced as [dh, tok]; scores are computed transposed
   (S^T = k^T q) so softmax's k-reduction lands on matmul-friendly axes.
 - softmax: exp on the scalar engine (scale folded in); the denominator is
   a pairwise fp16 tree-sum of the exp tiles on DVE plus one all-ones
   matmul partition-reduce; normalize via fast approximate reciprocal.
 - The PE is kept at full duty through phase B by interleaving one extra
   matmul per kt slot: o-proj matmuls of the previous token stripe (and,
   for the first stripe, the deferred stripe-3 q-projection chains). The
   last kt-pair's PV matmuls are emitted after the filler flush so the
   trailing exp latency is hidden at stripe boundaries.
 - Output is fp16 (host upcasts and sums partials), written with paired
   (2x128-row) DMA dispatches; the last stripe's softmax denominators are
   emitted per-head so the tail o-proj burst starts as early as possible.
"""

import sys

if "/opt/trn_rl_repo" not in sys.path:
    sys.path.insert(0, "/opt/trn_rl_repo")

import numpy as np

import concourse.bass as bass
import concourse.mybir as mybir
import concourse.tile as tile
from concourse.tile import add_dep_helper
from concourse import bacc

B, N, HID, H = 2, 2048, 2048, 16
DH = 128
HPC = 4  # heads per core
P = 128
F16 = mybir.dt.float16
F32 = mybir.dt.float32
SCALE = 1.0 / float(np.sqrt(DH))

_NC_CACHE = [None]


def build_nc():
    nc = bacc.Bacc(None, target_bir_lowering=False)

    KT = HID // P  # 16 contraction tiles over hid
    NT = N // P  # 16 token tiles
    NS = N // 512  # 4 token stripes
    MQK = 2 * HPC  # 8 output dim-tiles for q+k

    xT = nc.dram_tensor("xT", [HID, N], F16, kind="ExternalInput")
    # wqk blocked on host: [P, MQK, KT, P]; wqkB[p, m, kt, c] = w[m*P+c, kt*P+p]
    wqkB = nc.dram_tensor("wqkB", [P, MQK * KT * P], F16, kind="ExternalInput")
    wvT = nc.dram_tensor("wvT", [HID, HPC * DH], F16, kind="ExternalInput")
    woT = nc.dram_tensor("woT", [HPC * DH, HID], F16, kind="ExternalInput")
    cosT = nc.dram_tensor("cosT", [DH, N], F32, kind="ExternalInput")
    sinT = nc.dram_tensor("sinT", [DH, N], F32, kind="ExternalInput")
    outT = nc.dram_tensor("outT", [HID, N], F16, kind="ExternalOutput")

    mult = mybir.AluOpType.mult
    add = mybir.AluOpType.add
    Exp = mybir.ActivationFunctionType.Exp

    wqkB4 = wqkB[:].rearrange("p (m kt c) -> p m kt c", m=MQK, kt=KT)

    def x_src(rows, s):
        # xT rows [rows.start:rows.stop], stripe s columns, as [p, kt, n]
        return xT[rows, s * 512 : (s + 1) * 512].rearrange(
            "(kt p) n -> p kt n", p=P
        )

    with tile.TileContext(nc) as tc:
        with (
            tc.tile_pool(name="const", bufs=1) as const,
            tc.tile_pool(name="persist", bufs=1) as persist,
        ):
            warm_src = const.tile([P, 512], F16, tag="warmsrc")
            nc.vector.memset(warm_src[:], 0.0)
            ones_sb = const.tile([P, P], F16, tag="ones")

            # -------- startup loads: few big dispatches on two queues ------
            wqk_sb = const.tile([P, MQK, KT, P], F16, tag="wqk")
            wqk_flat = wqk_sb[:].rearrange("p m kt c -> p (m kt c)")
            cos_sb = const.tile([P, N], F32, tag="cos")
            sin_sb = const.tile([P, N], F32, tag="sin")
            # Startup loads: one ordered dispatch stream on the sync queue —
            # ring back-pressure makes transfers follow dispatch order, so
            # this sequence is the consumption order of phase A.
            nc.sync.dma_start(wqk_sb[:, 0, 0:4, :], wqkB4[:, 0, 0:4, :])
            nc.vector.memset(ones_sb[:], 1.0)

            # persistent intermediates
            qk_tiles = [
                persist.tile([P, N], F16, tag=f"qk{m}", name=f"qk{m}")
                for m in range(MQK)
            ]
            v_sb = persist.tile([P, NT, HPC * DH], F16, tag="v")

            # ---------------- Phase A: QKV + RoPE ----------------
            # (stripe 3's q chains are deferred into phase B's filler slots)
            with (
                tc.tile_pool(name="aphase", bufs=2) as aphase,
                tc.tile_pool(name="psumA", bufs=3, space="PSUM") as psumA,
            ):
                # x stripes 0-2 die with this pool; stripe 3 persists for the
                # phase-B q3 filler chains.
                x_stripes = [
                    (persist if s == NS - 1 else aphase).tile(
                        [P, KT, 512], F16, tag=f"xs{s}", name=f"xs{s}", bufs=1
                    )
                    for s in range(NS)
                ]
                wv_sb = aphase.tile([P, KT, HPC * DH], F16, tag="wv", bufs=1)
                # remaining startup loads: ONE ordered stream on the sync
                # queue — ring back-pressure makes transfers follow dispatch
                # order, and the order below is phase-A consumption order.
                # (Splitting across two queues halves each stream's share of
                # the ~360GB/s per-core HBM budget and starves the critical
                # path — measured 16us slower.)
                nc.sync.dma_start(x_stripes[0][:, 0:4, :], x_src(slice(0, 4 * P), 0))
                nc.sync.dma_start(
                    x_stripes[0][:, 4:10, :], x_src(slice(4 * P, 10 * P), 0)
                )
                nc.sync.dma_start(
                    wqk_flat[:, 4 * P : 2 * KT * P], wqkB[:, 4 * P : 2 * KT * P]
                )
                nc.sync.dma_start(
                    x_stripes[0][:, 10:KT, :], x_src(slice(10 * P, HID), 0)
                )
                nc.sync.dma_start(cos_sb[:, 0:512], cosT[:, 0:512])
                nc.sync.dma_start(sin_sb[:, 0:512], sinT[:, 0:512])
                nc.sync.dma_start(
                    wqk_flat[:, 2 * KT * P : 4 * KT * P],
                    wqkB[:, 2 * KT * P : 4 * KT * P],
                )
                nc.sync.dma_start(
                    wqk_flat[:, 4 * KT * P :], wqkB[:, 4 * KT * P :]
                )
                nc.sync.dma_start(
                    wv_sb[:], wvT[:].rearrange("(kt p) m -> p kt m", p=P)
                )
                nc.sync.dma_start(x_stripes[1][:], x_src(slice(0, HID), 1))
                nc.sync.dma_start(cos_sb[:, 512:N], cosT[:, 512:N])
                nc.sync.dma_start(sin_sb[:, 512:N], sinT[:, 512:N])
                nc.sync.dma_start(x_stripes[2][:], x_src(slice(0, HID), 2))
                nc.sync.dma_start(x_stripes[3][:], x_src(slice(0, HID), 3))

                warm_ps = psumA.tile([P, 512], F32, tag="warm", bufs=1)
                for _ in range(6):
                    nc.tensor.matmul(
                        warm_ps[:], warm_src[:, 0:P], warm_src[:],
                        start=True, stop=True,
                    )

                def emit_qk_chain(s, m, x_tiles, fast_drain=False):
                    # fast_drain (last stripe): route the rot copies to the
                    # idle gpsimd engine so the psum bank is read (and freed
                    # for phase B's tiles) without queuing behind the
                    # activation engine's copy backlog.
                    sl = slice(s * 512, (s + 1) * 512)
                    ps = psumA.tile([P, 512], F32, tag="ps", name="ps")
                    for kt in range(KT):
                        nc.tensor.matmul(
                            ps[:],
                            wqk_sb[:, m, kt, :],
                            x_tiles[kt][:],
                            start=(kt == 0),
                            stop=(kt == KT - 1),
                        )
                    # RoPE: out = ps*cos + rot(ps)*sin_signed
                    rot = aphase.tile([P, 512], F32, tag="rot", name="rot")
                    nc.scalar.copy(rot[0:64, :], ps[64:128, :])
                    nc.scalar.copy(rot[64:128, :], ps[0:64, :])
                    t1 = aphase.tile([P, 512], F32, tag="t1", name="t1")
                    nc.vector.tensor_tensor(t1[:], rot[:], sin_sb[:, sl], mult)
                    t2 = aphase.tile([P, 512], F32, tag="t2", name="t2")
                    nc.vector.tensor_tensor(t2[:], ps[:], cos_sb[:, sl], mult)
                    nc.vector.tensor_tensor(qk_tiles[m][:, sl], t1[:], t2[:], add)

                def emit_v_chain(s, tt, x_tiles, fast_drain=False):
                    tok = s * 4 + tt
                    psv = psumA.tile([P, 512], F32, tag="psv", bufs=2, name="psv")
                    for kt in range(KT):
                        nc.tensor.matmul(
                            psv[:],
                            x_tiles[kt][:, tt * P : (tt + 1) * P],
                            wv_sb[:, kt, :],
                            start=(kt == 0),
                            stop=(kt == KT - 1),
                        )
                    if fast_drain:
                        # DVE is idle during the last stripe's v chains; the
                        # activation queue would serialize these behind the
                        # RoPE rot-copy backlog right before phase B needs
                        # the psum banks back.
                        nc.vector.tensor_copy(v_sb[:, tok, :], psv[:])
                    else:
                        nc.scalar.copy(v_sb[:, tok, :], psv[:])

                for s in range(NS):
                    x_tiles = [x_stripes[s][:, kt, :] for kt in range(KT)]
                    last = s == NS - 1
                    m_list = range(MQK) if not last else range(HPC, MQK)
                    for m in m_list:
                        emit_qk_chain(s, m, x_tiles, fast_drain=last)
                    for tt in range(4):
                        emit_v_chain(s, tt, x_tiles, fast_drain=last)

            # ---------------- Phases B (attention) + C (o-proj) ----------
            with tc.tile_pool(name="late", bufs=3) as late:
                wo_sb = late.tile([P, HPC, HID], F16, tag="wo", bufs=1)
                nc.scalar.dma_start(
                    wo_sb[:], woT[:].rearrange("(ht p) o -> p ht o", p=P)
                )
                attn_tiles = [
                    late.tile([P, N], F16, tag=f"attn{h}", name=f"attn{h}", bufs=1)
                    for h in range(HPC)
                ]

                with tc.tile_pool(name="psumB", bufs=2, space="PSUM") as psumB:
                    pending = []  # deferred denominator/normalize closure args

                    def emit_tree(accs):
                        # pairwise fp16 tree-sum of the 8 exp accumulators;
                        # emitted at head end so the DVE work is done before
                        # the denominator matmul lands on the (in-order) PE
                        # queue at the pop point.
                        stride = 1
                        while stride < 8:
                            for i in range(0, 8, 2 * stride):
                                nc.vector.tensor_tensor(
                                    accs[i][:], accs[i][:], accs[i + stride][:], add
                                )
                            stride *= 2

                    def emit_denorm(accs, outp, h, qsl):
                        den = psumB.tile([P, 512], F32, tag="cps", bufs=2, name="den")
                        nc.tensor.matmul(
                            den[:], ones_sb[:], accs[0][:], start=True, stop=True
                        )
                        rec = late.tile([P, 512], F32, tag="rec", bufs=2)
                        nc.vector.reciprocal_approx_fast(rec[:], den[:])
                        nc.vector.tensor_tensor(
                            attn_tiles[h][:, qsl], outp[:], rec[:], mult
                        )

                    def make_c_fillers(ts, tag="cps", co=None):
                        # one closure per o-proj matmul for token stripe ts,
                        # spread through the next stripe's kt slots to keep
                        # the PE at full duty. Output DMAs are paired over
                        # two consecutive ho blocks (one dispatch per pair).
                        tsl = slice(ts * 512, (ts + 1) * 512)
                        state = {}
                        if co is None:
                            co = {}
                        fillers = []

                        def mk(ho, hi, tag="cps"):
                            def emit():
                                if hi == 0:
                                    state[ho] = psumB.tile(
                                        [P, 512], F32, tag=tag, bufs=2, name="cps"
                                    )
                                nc.tensor.matmul(
                                    state[ho][:],
                                    wo_sb[:, hi, ho * P : (ho + 1) * P],
                                    attn_tiles[hi][:, tsl],
                                    start=(hi == 0),
                                    stop=(hi == HPC - 1),
                                )
                                if hi == HPC - 1:
                                    if ho % 2 == 0:
                                        co[ho // 2] = late.tile(
                                            [P, 2, 512], F16, tag="co", bufs=3,
                                            name="co",
                                        )
                                    ob = co[ho // 2]
                                    nc.vector.tensor_copy(
                                        ob[:, ho % 2, :], state[ho][:]
                                    )
                                    if ho % 2 == 1:
                                        nc.sync.dma_start(
                                            outT[
                                                (ho - 1) * P : (ho + 1) * P, tsl
                                            ].rearrange("(i p) n -> p i n", p=P),
                                            ob[:],
                                        )
                            return emit

                        for ho in range(HID // P):
                            for hi in range(HPC):
                                fillers.append(mk(ho, hi, tag=tag))
                        return fillers

                    def make_q3_fillers():
                        # deferred stripe-3 q-projection chains (m=0..3),
                        # one matmul per filler slot + RoPE at chain end.
                        s3 = NS - 1
                        sl3 = slice(s3 * 512, (s3 + 1) * 512)
                        x3 = x_stripes[s3]
                        state = {}
                        fillers = []

                        def mk(m, kt):
                            def emit():
                                if kt == 0:
                                    state[m] = psumB.tile(
                                        [P, 512], F32, tag="cps", bufs=2, name="q3ps"
                                    )
                                nc.tensor.matmul(
                                    state[m][:],
                                    wqk_sb[:, m, kt, :],
                                    x3[:, kt, :],
                                    start=(kt == 0),
                                    stop=(kt == KT - 1),
                                )
                                if kt == KT - 1:
                                    ps = state[m]
                                    rot = late.tile([P, 512], F32, tag="rot3", bufs=2)
                                    nc.vector.tensor_copy(rot[0:64, :], ps[64:128, :])
                                    nc.vector.tensor_copy(rot[64:128, :], ps[0:64, :])
                                    t1 = late.tile([P, 512], F32, tag="t13", bufs=2)
                                    nc.vector.tensor_tensor(
                                        t1[:], rot[:], sin_sb[:, sl3], mult
                                    )
                                    t2 = late.tile([P, 512], F32, tag="t23", bufs=2)
                                    nc.vector.tensor_tensor(
                                        t2[:], ps[:], cos_sb[:, sl3], mult
                                    )
                                    nc.vector.tensor_tensor(
                                        qk_tiles[m][:, sl3], t1[:], t2[:], add
                                    )
                            return emit

                        for m in range(HPC):
                            for kt in range(KT):
                                fillers.append(mk(m, kt))
                        return fillers

                    def emit_pv_acc(pt, outp, accs, h, kt):
                        pvs = []
                        for half in range(2):
                            k2 = kt - 1 + half
                            pv = nc.tensor.matmul(
                                outp[:],
                                v_sb[:, k2, h * DH : (h + 1) * DH],
                                pt[:, half * 512 : (half + 1) * 512],
                                start=(k2 == 0),
                                stop=(k2 == NT - 1),
                            )
                            pvs.append(pv)
                        acc = late.tile(
                            [P, 512], F16, tag="acc", bufs=10, name="acc"
                        )
                        ai = nc.vector.tensor_tensor(
                            acc[:], pt[:, 0:512], pt[:, 512:1024], add
                        )
                        for pv in pvs:
                            add_dep_helper(ai.ins, pv.ins,
                                           reason="pt SBUF contention")
                        accs.append(acc)

                    for qs in range(NS):
                        last_stripe = qs == NS - 1
                        qsl = slice(qs * 512, (qs + 1) * 512)
                        fillers = (
                            make_c_fillers(qs - 1) if qs > 0 else make_q3_fillers()
                        )
                        fi = 0
                        for h in range(HPC):
                            last_head = h == HPC - 1
                            kT_tile = qk_tiles[HPC + h]
                            qT_tile = qk_tiles[h]
                            outp = psumB.tile([P, 512], F32, tag="po", bufs=2)
                            accs = []
                            st2 = None
                            deferred_pv = None
                            for kt in range(NT):
                                if kt % 2 == 0:
                                    st2 = psumB.tile(
                                        [P, 1024], F32, tag="st", bufs=2, name="st2"
                                    )
                                nc.tensor.matmul(
                                    st2[:, (kt % 2) * 512 : (kt % 2 + 1) * 512],
                                    kT_tile[:, kt * P : (kt + 1) * P],
                                    qT_tile[:, qsl],
                                    start=True,
                                    stop=True,
                                )
                                pt = None
                                if kt % 2 == 1:
                                    # one 1024-wide exp per ST pair (halves the
                                    # scalar-engine instruction/sem count)
                                    pt = late.tile([P, 1024], F16, tag="pt", bufs=3)
                                    nc.scalar.activation(
                                        pt[:], st2[:], Exp, scale=SCALE
                                    )
                                if kt == 1 and pending:
                                    emit_denorm(*pending.pop())
                                # reserve 5 fillers for the deferred-PV flush
                                # at the stripe end (covers the last exp's
                                # latency on the in-order PE queue)
                                if (h > 0 or kt >= 2) and fi < len(fillers) - 5:
                                    fillers[fi]()
                                    fi += 1
                                if pt is not None:
                                    if last_head and kt == NT - 1:
                                        # defer the trailing PVs past the filler
                                        # flush so the exp latency is hidden
                                        deferred_pv = pt
                                    else:
                                        emit_pv_acc(pt, outp, accs, h, kt)
                            if deferred_pv is not None:
                                while fi < len(fillers):
                                    fillers[fi]()
                                    fi += 1
                                emit_pv_acc(deferred_pv, outp, accs, h, NT - 1)
                            emit_tree(accs)
                            if last_stripe:
                                # immediate denorm so the tail o-proj burst
                                # for this stripe starts as early as possible
                                emit_denorm(accs, outp, h, qsl)
                            else:
                                pending.append((accs, outp, h, qsl))
                        while fi < len(fillers):
                            fillers[fi]()
                            fi += 1
                    # final stripe's o-proj: rotate the tail burst over three
                    # psum tags (st is free once the last scores are done)
                    # for deeper pipelining
                    tail_f = []
                    tail_co = {}
                    for tag in ("cps", "po", "st"):
                        tail_f.append(make_c_fillers(NS - 1, tag=tag, co=tail_co))
                    for g in range(HID // P):
                        for hi in range(HPC):
                            tail_f[g % 3][g * HPC + hi]()

    nc.finalize()
    return nc


def get_nc():
    if _NC_CACHE[0] is None:
        _NC_CACHE[0] = build_nc()
    return _NC_CACHE[0]


def make_in_maps(hidden_states, cos, sin, w_qkv, w_o):
    """Build the 8 per-core input maps (host-side shard + transpose + cast)."""
    hidden_states = np.asarray(hidden_states, dtype=np.float32)
    cos = np.asarray(cos, dtype=np.float32)
    sin = np.asarray(sin, dtype=np.float32)
    w_qkv = np.asarray(w_qkv, dtype=np.float32)
    w_o = np.asarray(w_o, dtype=np.float32)

    KT = HID // P
    MQK = 2 * HPC

    cosT = np.ascontiguousarray(cos.T)  # [DH, N]
    sinT_signed = np.ascontiguousarray(
        np.concatenate([-sin.T[: DH // 2], sin.T[DH // 2 :]], axis=0)
    )

    xT = [
        np.ascontiguousarray(hidden_states[b].T).astype(np.float16)
        for b in range(B)
    ]

    in_maps = []
    for c in range(8):
        b, g = divmod(c, 4)
        qrows = slice(g * HPC * DH, (g + 1) * HPC * DH)
        krows = slice(HID + g * HPC * DH, HID + (g + 1) * HPC * DH)
        vrows = slice(2 * HID + g * HPC * DH, 2 * HID + (g + 1) * HPC * DH)
        # blocked wqk: wqkB[p, m, kt, c] = wqk_rows[m*P+c, kt*P+p]
        wqk_rows = np.concatenate([w_qkv[qrows], w_qkv[krows]], axis=0)
        wqkB = (
            wqk_rows.reshape(MQK, P, KT, P)
            .transpose(3, 0, 2, 1)
            .reshape(P, MQK * KT * P)
            .astype(np.float16)
        )
        wvT = w_qkv[vrows].T.astype(np.float16)
        woT = w_o[:, g * HPC * DH : (g + 1) * HPC * DH].T.astype(np.float16)
        in_maps.append(
            {
                "xT": xT[b],
                "wqkB": np.ascontiguousarray(wqkB),
                "wvT": np.ascontiguousarray(wvT),
                "woT": np.ascontiguousarray(woT),
                "cosT": cosT,
                "sinT": sinT_signed,
            }
        )
    return in_maps


def assemble_output(results):
    """Sum the 4 o-proj fp16 partials per batch (in fp32) and transpose."""
    out = np.zeros((B, N, HID), dtype=np.float32)
    for c, res in enumerate(results):
        b = c // 4
        out[b] += res["outT"].T.astype(np.float32)
    return out


def kernel(hidden_states, cos, sin, w_qkv, w_o):
    from concourse.bass_utils import run_bass_kernel_spmd

    nc = get_nc()
    in_maps = make_in_maps(hidden_states, cos, sin, w_qkv, w_o)
    res = run_bass_kernel_spmd(nc, in_maps, core_ids=list(range(8)))
    return assemble_output(res.results)



# revision 2
# speedup vs baseline: 1.0150x; 1.0150x over previous
"""Multi-head attention (QKV proj + RoPE + softmax attention + o-proj) on 8
Trainium2 NeuronCores.

Sharding: data-parallel over batch (B=2) x tensor-parallel over heads
(16 heads -> 4 groups of 4). Core c handles batch c//4, heads 4*(c%4)..+4.
qkv_proj is column-parallel, o_proj row-parallel; each core returns a
partial o-proj output (fp16) and the host sums the 4 partials per batch.

All matmuls run in fp16 (full PE speed) with fp32 PSUM accumulation.

Schedule notes (per core):
 - Phase A stripe 0 runs kt-OUTER: all 8 q/k chains accumulate in 8
   dedicated PSUM banks, one matmul per (kt, m), so the PE consumes x/w
   tiles in DMA-arrival order. The DMA stream interleaves wqk/x0 by
   kt-pair, so the first matmul fires ~1.5us in and the PE never starves
   -> HAM warms at ~5us instead of ~24us.
 - RoPE drains are split: bank-freeing reads (rot copies on ACT, t2 on
   DVE) are emitted at chain end; the t1/add finishes are deferred into
   the v-chain window so stripe-0 banks free at ACT pace.
 - q,k are produced as [dh, tok]; scores are computed transposed
   (S^T = k^T q) so softmax's k-reduction lands on matmul-friendly axes.
 - softmax: exp on the scalar engine (scale folded in); the denominator is
   a pairwise fp16 tree-sum of the exp tiles on DVE plus one all-ones
   matmul partition-reduce; normalize via fast approximate reciprocal.
 - The PE is kept at full duty through phase B by interleaving one extra
   matmul per kt slot: o-proj matmuls of the previous token stripe (and,
   for the first stripe, the deferred stripe-3 q-projection chains). The
   last kt-pair's PV matmuls are emitted after the filler flush so the
   trailing exp latency is hidden at stripe boundaries.
 - Output is fp16 (host upcasts and sums partials), written with paired
   (2x128-row) DMA dispatches; the last stripe's softmax denominators are
   emitted per-head so the tail o-proj burst starts as early as possible.
"""

import sys

if "/opt/trn_rl_repo" not in sys.path:
    sys.path.insert(0, "/opt/trn_rl_repo")

import numpy as np

import concourse.bass as bass
import concourse.mybir as mybir
import concourse.tile as tile
from concourse.tile import add_dep_helper
from concourse import bacc

B, N, HID, H = 2, 2048, 2048, 16
DH = 128
HPC = 4  # heads per core
P = 128
F16 = mybir.dt.float16
F32 = mybir.dt.float32
SCALE = 1.0 / float(np.sqrt(DH))

_NC_CACHE = [None]


def build_nc():
    nc = bacc.Bacc(None, target_bir_lowering=False)

    KT = HID // P  # 16 contraction tiles over hid
    NT = N // P  # 16 token tiles
    NS = N // 512  # 4 token stripes
    MQK = 2 * HPC  # 8 output dim-tiles for q+k

    xT = nc.dram_tensor("xT", [HID, N], F16, kind="ExternalInput")
    # wqk blocked on host, kt-major: wqkB[p, kt, m, c] = w[m*P+c, kt*P+p]
    wqkB = nc.dram_tensor("wqkB", [P, KT * MQK * P], F16, kind="ExternalInput")
    wvT = nc.dram_tensor("wvT", [HID, HPC * DH], F16, kind="ExternalInput")
    woT = nc.dram_tensor("woT", [HPC * DH, HID], F16, kind="ExternalInput")
    cosT = nc.dram_tensor("cosT", [DH, N], F16, kind="ExternalInput")
    sinT = nc.dram_tensor("sinT", [DH, N], F16, kind="ExternalInput")
    outT = nc.dram_tensor("outT", [HID, N], F16, kind="ExternalOutput")

    mult = mybir.AluOpType.mult
    add = mybir.AluOpType.add
    Exp = mybir.ActivationFunctionType.Exp

    wqkB4 = wqkB[:].rearrange("p (kt m c) -> p kt m c", kt=KT, m=MQK)

    def x_src(s, kt0, kt1):
        # xT rows for kt tiles [kt0, kt1), stripe s columns, as [p, kt, n]
        return xT[kt0 * P : kt1 * P, s * 512 : (s + 1) * 512].rearrange(
            "(kt p) n -> p kt n", p=P
        )

    with tile.TileContext(nc) as tc:
        with (
            tc.tile_pool(name="const", bufs=1) as const,
            tc.tile_pool(name="persist", bufs=1) as persist,
        ):
            warm_src = const.tile([P, 512], F16, tag="warmsrc")
            nc.vector.memset(warm_src[:], 0.0)
            ones_sb = const.tile([P, P], F16, tag="ones")

            wqk_sb = const.tile([P, KT, MQK, P], F16, tag="wqk")
            cos_sb = const.tile([P, N], F16, tag="cos")
            sin_sb = const.tile([P, N], F16, tag="sin")
            nc.vector.memset(ones_sb[:], 1.0)

            # persistent intermediates
            qk_tiles = [
                persist.tile([P, N], F16, tag=f"qk{m}", name=f"qk{m}")
                for m in range(MQK)
            ]
            v_sb = persist.tile([P, NT, HPC * DH], F16, tag="v")

            # ---------------- Phase A: QKV + RoPE ----------------
            # (stripe 3's q chains are deferred into phase B's filler slots)
            with (
                tc.tile_pool(name="aphase", bufs=2) as aphase,
                tc.tile_pool(name="psumA", bufs=1, space="PSUM") as psumA,
            ):
                # x stripes 0-2 die with this pool; stripe 3 persists for the
                # phase-B q3 filler chains.
                x_stripes = [
                    (persist if s == NS - 1 else aphase).tile(
                        [P, KT, 512], F16, tag=f"xs{s}", name=f"xs{s}", bufs=1
                    )
                    for s in range(NS)
                ]
                wv_sb = aphase.tile([P, KT, HPC * DH], F16, tag="wv", bufs=1)
                # Startup loads: ONE ordered stream on the sync queue — ring
                # back-pressure makes transfers follow dispatch order, and
                # the order below is phase-A consumption order.  Stripe 0 is
                # interleaved by kt-PAIR (wqk then x) to match the kt-outer
                # matmul order of stripe 0.
                for kt0 in range(0, KT, 2):
                    nc.sync.dma_start(
                        wqk_sb[:, kt0 : kt0 + 2, :, :],
                        wqkB4[:, kt0 : kt0 + 2, :, :],
                    )
                    nc.sync.dma_start(
                        x_stripes[0][:, kt0 : kt0 + 2, :],
                        x_src(0, kt0, kt0 + 2),
                    )
                nc.sync.dma_start(cos_sb[:, 0:512], cosT[:, 0:512])
                nc.sync.dma_start(sin_sb[:, 0:512], sinT[:, 0:512])
                nc.sync.dma_start(
                    wv_sb[:], wvT[:].rearrange("(kt p) m -> p kt m", p=P)
                )
                nc.sync.dma_start(x_stripes[1][:], x_src(1, 0, KT))
                nc.sync.dma_start(cos_sb[:, 512:N], cosT[:, 512:N])
                nc.sync.dma_start(sin_sb[:, 512:N], sinT[:, 512:N])
                nc.sync.dma_start(x_stripes[2][:], x_src(2, 0, KT))
                nc.sync.dma_start(x_stripes[3][:], x_src(3, 0, KT))

                # 8 dedicated PSUM banks for the whole of phase A:
                # qk chain m -> bank m; v chain tt -> bank tt (reused).
                ps_m = [
                    psumA.tile([P, 512], F32, tag=f"pa{m}", name=f"pa{m}")
                    for m in range(MQK)
                ]

                # HAM warmup: short matmuls while the first DMAs land.
                for _ in range(8):
                    nc.tensor.matmul(
                        ps_m[0][:, 0:256], warm_src[:, 0:P], warm_src[:, 0:256],
                        start=True, stop=True,
                    )

                def emit_rope_drain(s, m, ps):
                    # bank-freeing reads of ps only (rot on ACT, t2 on DVE);
                    # rot/t2 are kept in fp16 so stripe 0 can hold all 8.
                    sl = slice(s * 512, (s + 1) * 512)
                    rot = aphase.tile([P, 512], F16, tag="rot", bufs=9,
                                      name="rot")
                    nc.scalar.copy(rot[0:64, :], ps[64:128, :])
                    nc.scalar.copy(rot[64:128, :], ps[0:64, :])
                    t2 = aphase.tile([P, 512], F16, tag="t2", bufs=9, name="t2")
                    nc.vector.tensor_tensor(t2[:], ps[:], cos_sb[:, sl], mult)
                    return rot, t2

                def emit_rope_finish(s, m, rot, t2):
                    sl = slice(s * 512, (s + 1) * 512)
                    t1 = aphase.tile([P, 512], F16, tag="t1", bufs=2, name="t1")
                    nc.vector.tensor_tensor(t1[:], rot[:], sin_sb[:, sl], mult)
                    nc.vector.tensor_tensor(qk_tiles[m][:, sl], t1[:], t2[:], add)

                def emit_qk_chain(s, m):
                    # m-outer chain for stripes 1+ (data already resident)
                    ps = ps_m[m]
                    for kt in range(KT):
                        nc.tensor.matmul(
                            ps[:],
                            wqk_sb[:, kt, m, :],
                            x_stripes[s][:, kt, :],
                            start=(kt == 0),
                            stop=(kt == KT - 1),
                        )
                    rot, t2 = emit_rope_drain(s, m, ps)
                    emit_rope_finish(s, m, rot, t2)

                def emit_v_chain(s, tt, fast_drain=False):
                    tok = s * 4 + tt
                    psv = ps_m[tt]
                    for kt in range(KT):
                        nc.tensor.matmul(
                            psv[:],
                            x_stripes[s][:, kt, tt * P : (tt + 1) * P],
                            wv_sb[:, kt, :],
                            start=(kt == 0),
                            stop=(kt == KT - 1),
                        )
                    if fast_drain:
                        # DVE is idle during the last stripe's v chains; the
                        # activation queue would serialize these behind the
                        # RoPE rot-copy backlog right before phase B needs
                        # the psum banks back.
                        nc.vector.tensor_copy(v_sb[:, tok, :], psv[:])
                    else:
                        nc.scalar.copy(v_sb[:, tok, :], psv[:])

                # ---- stripe 0: kt-outer so the PE follows the DMA stream
                pend_rope = []
                for kt in range(KT):
                    for m in range(MQK):
                        nc.tensor.matmul(
                            ps_m[m][:],
                            wqk_sb[:, kt, m, :],
                            x_stripes[0][:, kt, :],
                            start=(kt == 0),
                            stop=(kt == KT - 1),
                        )
                        if kt == KT - 1:
                            pend_rope.append(
                                (m, emit_rope_drain(0, m, ps_m[m]))
                            )
                for tt in range(4):
                    emit_v_chain(0, tt)
                    # rope finishes ride the idle DVE during the v window
                    for _ in range(2):
                        if pend_rope:
                            m, (rot, t2) = pend_rope.pop(0)
                            emit_rope_finish(0, m, rot, t2)

                # ---- stripes 1-3: m-outer chains
                for s in range(1, NS):
                    last = s == NS - 1
                    m_list = range(MQK) if not last else range(HPC, MQK)
                    for m in m_list:
                        emit_qk_chain(s, m)
                    for tt in range(4):
                        emit_v_chain(s, tt, fast_drain=last)

            # ---------------- Phases B (attention) + C (o-proj) ----------
            with tc.tile_pool(name="late", bufs=3) as late:
                wo_sb = late.tile([P, HPC, HID], F16, tag="wo", bufs=1)
                nc.sync.dma_start(
                    wo_sb[:], woT[:].rearrange("(ht p) o -> p ht o", p=P)
                )
                attn_tiles = [
                    late.tile([P, N], F16, tag=f"attn{h}", name=f"attn{h}", bufs=1)
                    for h in range(HPC)
                ]

                with tc.tile_pool(name="psumB", bufs=2, space="PSUM") as psumB:
                    pending = []  # deferred denominator/normalize closure args

                    def emit_tree(accs):
                        # pairwise fp16 tree-sum of the 8 exp accumulators;
                        # emitted at head end so the DVE work is done before
                        # the denominator matmul lands on the (in-order) PE
                        # queue at the pop point.
                        stride = 1
                        while stride < 8:
                            for i in range(0, 8, 2 * stride):
                                nc.vector.tensor_tensor(
                                    accs[i][:], accs[i][:], accs[i + stride][:], add
                                )
                            stride *= 2

                    def emit_denorm(accs, outp, h, qsl):
                        den = psumB.tile([P, 512], F32, tag="cps", bufs=2, name="den")
                        nc.tensor.matmul(
                            den[:], ones_sb[:], accs[0][:], start=True, stop=True
                        )
                        rec = late.tile([P, 512], F32, tag="rec", bufs=2)
                        nc.vector.reciprocal_approx_fast(rec[:], den[:])
                        nc.vector.tensor_tensor(
                            attn_tiles[h][:, qsl], outp[:], rec[:], mult
                        )

                    def make_c_fillers(ts, tag="cps", co=None):
                        # one closure per o-proj matmul for token stripe ts,
                        # spread through the next stripe's kt slots to keep
                        # the PE at full duty. Output DMAs are paired over
                        # two consecutive ho blocks (one dispatch per pair).
                        tsl = slice(ts * 512, (ts + 1) * 512)
                        state = {}
                        if co is None:
                            co = {}
                        fillers = []

                        def mk(ho, hi, tag="cps"):
                            def emit():
                                if hi == 0:
                                    state[ho] = psumB.tile(
                                        [P, 512], F32, tag=tag, bufs=2, name="cps"
                                    )
                                nc.tensor.matmul(
                                    state[ho][:],
                                    wo_sb[:, hi, ho * P : (ho + 1) * P],
                                    attn_tiles[hi][:, tsl],
                                    start=(hi == 0),
                                    stop=(hi == HPC - 1),
                                )
                                if hi == HPC - 1:
                                    if ho % 2 == 0:
                                        co[ho // 2] = late.tile(
                                            [P, 2, 512], F16, tag="co", bufs=3,
                                            name="co",
                                        )
                                    ob = co[ho // 2]
                                    nc.vector.tensor_copy(
                                        ob[:, ho % 2, :], state[ho][:]
                                    )
                                    if ho % 2 == 1:
                                        nc.sync.dma_start(
                                            outT[
                                                (ho - 1) * P : (ho + 1) * P, tsl
                                            ].rearrange("(i p) n -> p i n", p=P),
                                            ob[:],
                                        )
                            return emit

                        for ho in range(HID // P):
                            for hi in range(HPC):
                                fillers.append(mk(ho, hi, tag=tag))
                        return fillers

                    def make_q3_fillers():
                        # deferred stripe-3 q-projection chains (m=0..3),
                        # one matmul per filler slot + RoPE at chain end.
                        s3 = NS - 1
                        sl3 = slice(s3 * 512, (s3 + 1) * 512)
                        x3 = x_stripes[s3]
                        state = {}
                        fillers = []

                        def mk(m, kt):
                            def emit():
                                if kt == 0:
                                    state[m] = psumB.tile(
                                        [P, 512], F32, tag="cps", bufs=2, name="q3ps"
                                    )
                                nc.tensor.matmul(
                                    state[m][:],
                                    wqk_sb[:, kt, m, :],
                                    x3[:, kt, :],
                                    start=(kt == 0),
                                    stop=(kt == KT - 1),
                                )
                                if kt == KT - 1:
                                    ps = state[m]
                                    rot = late.tile([P, 512], F32, tag="rot3", bufs=2)
                                    nc.vector.tensor_copy(rot[0:64, :], ps[64:128, :])
                                    nc.vector.tensor_copy(rot[64:128, :], ps[0:64, :])
                                    t1 = late.tile([P, 512], F32, tag="t13", bufs=2)
                                    nc.vector.tensor_tensor(
                                        t1[:], rot[:], sin_sb[:, sl3], mult
                                    )
                                    t2 = late.tile([P, 512], F32, tag="t23", bufs=2)
                                    nc.vector.tensor_tensor(
                                        t2[:], ps[:], cos_sb[:, sl3], mult
                                    )
                                    nc.vector.tensor_tensor(
                                        qk_tiles[m][:, sl3], t1[:], t2[:], add
                                    )
                            return emit

                        for m in range(HPC):
                            for kt in range(KT):
                                fillers.append(mk(m, kt))
                        return fillers

                    def emit_pv_acc(pt, outp, accs, h, kt):
                        pvs = []
                        for half in range(2):
                            k2 = kt - 1 + half
                            pv = nc.tensor.matmul(
                                outp[:],
                                v_sb[:, k2, h * DH : (h + 1) * DH],
                                pt[:, half * 512 : (half + 1) * 512],
                                start=(k2 == 0),
                                stop=(k2 == NT - 1),
                            )
                            pvs.append(pv)
                        acc = late.tile(
                            [P, 512], F16, tag="acc", bufs=10, name="acc"
                        )
                        ai = nc.vector.tensor_tensor(
                            acc[:], pt[:, 0:512], pt[:, 512:1024], add
                        )
                        for pv in pvs:
                            add_dep_helper(ai.ins, pv.ins,
                                           reason="pt SBUF contention")
                        accs.append(acc)

                    for qs in range(NS):
                        last_stripe = qs == NS - 1
                        qsl = slice(qs * 512, (qs + 1) * 512)
                        fillers = (
                            make_c_fillers(qs - 1) if qs > 0 else make_q3_fillers()
                        )
                        fi = 0
                        for h in range(HPC):
                            last_head = h == HPC - 1
                            kT_tile = qk_tiles[HPC + h]
                            qT_tile = qk_tiles[h]
                            outp = psumB.tile([P, 512], F32, tag="po", bufs=2)
                            accs = []
                            st2 = None
                            deferred_pv = None
                            for kt in range(NT):
                                if kt % 2 == 0:
                                    st2 = psumB.tile(
                                        [P, 1024], F32, tag="st", bufs=2, name="st2"
                                    )
                                nc.tensor.matmul(
                                    st2[:, (kt % 2) * 512 : (kt % 2 + 1) * 512],
                                    kT_tile[:, kt * P : (kt + 1) * P],
                                    qT_tile[:, qsl],
                                    start=True,
                                    stop=True,
                                )
                                pt = None
                                if kt % 2 == 1:
                                    # one 1024-wide exp per ST pair (halves the
                                    # scalar-engine instruction/sem count)
                                    pt = late.tile([P, 1024], F16, tag="pt", bufs=3)
                                    nc.scalar.activation(
                                        pt[:], st2[:], Exp, scale=SCALE
                                    )
                                if kt == 1 and pending:
                                    emit_denorm(*pending.pop())
                                # reserve 5 fillers for the deferred-PV flush
                                # at the stripe end (covers the last exp's
                                # latency on the in-order PE queue)
                                if (h > 0 or kt >= 2) and fi < len(fillers) - 5:
                                    fillers[fi]()
                                    fi += 1
                                if pt is not None:
                                    if last_head and kt == NT - 1:
                                        # defer the trailing PVs past the filler
                                        # flush so the exp latency is hidden
                                        deferred_pv = pt
                                    else:
                                        emit_pv_acc(pt, outp, accs, h, kt)
                            if deferred_pv is not None:
                                while fi < len(fillers):
                                    fillers[fi]()
                                    fi += 1
                                emit_pv_acc(deferred_pv, outp, accs, h, NT - 1)
                            emit_tree(accs)
                            if last_stripe:
                                # immediate denorm so the tail o-proj burst
                                # for this stripe starts as early as possible
                                emit_denorm(accs, outp, h, qsl)
                            else:
                                pending.append((accs, outp, h, qsl))
                        while fi < len(fillers):
                            fillers[fi]()
                            fi += 1
                    # final stripe's o-proj: rotate the tail burst over three
                    # psum tags (st is free once the last scores are done)
                    # for deeper pipelining
                    tail_f = []
                    tail_co = {}
                    for tag in ("cps", "po", "st"):
                        tail_f.append(make_c_fillers(NS - 1, tag=tag, co=tail_co))
                    for g in range(HID // P):
                        for hi in range(HPC):
                            tail_f[g % 3][g * HPC + hi]()

    nc.finalize()
    return nc


def get_nc():
    if _NC_CACHE[0] is None:
        _NC_CACHE[0] = build_nc()
    return _NC_CACHE[0]


def make_in_maps(hidden_states, cos, sin, w_qkv, w_o):
    """Build the 8 per-core input maps (host-side shard + transpose + cast)."""
    hidden_states = np.asarray(hidden_states, dtype=np.float32)
    cos = np.asarray(cos, dtype=np.float32)
    sin = np.asarray(sin, dtype=np.float32)
    w_qkv = np.asarray(w_qkv, dtype=np.float32)
    w_o = np.asarray(w_o, dtype=np.float32)

    KT = HID // P
    MQK = 2 * HPC

    cosT = np.ascontiguousarray(cos.T).astype(np.float16)  # [DH, N]
    sinT_signed = np.ascontiguousarray(
        np.concatenate([-sin.T[: DH // 2], sin.T[DH // 2 :]], axis=0)
    ).astype(np.float16)

    xT = [
        np.ascontiguousarray(hidden_states[b].T).astype(np.float16)
        for b in range(B)
    ]

    in_maps = []
    for c in range(8):
        b, g = divmod(c, 4)
        qrows = slice(g * HPC * DH, (g + 1) * HPC * DH)
        krows = slice(HID + g * HPC * DH, HID + (g + 1) * HPC * DH)
        vrows = slice(2 * HID + g * HPC * DH, 2 * HID + (g + 1) * HPC * DH)
        # blocked wqk, kt-major: wqkB[p, kt, m, c] = wqk_rows[m*P+c, kt*P+p]
        wqk_rows = np.concatenate([w_qkv[qrows], w_qkv[krows]], axis=0)
        wqkB = (
            wqk_rows.reshape(MQK, P, KT, P)
            .transpose(3, 2, 0, 1)
            .reshape(P, KT * MQK * P)
            .astype(np.float16)
        )
        wvT = w_qkv[vrows].T.astype(np.float16)
        woT = w_o[:, g * HPC * DH : (g + 1) * HPC * DH].T.astype(np.float16)
        in_maps.append(
            {
                "xT": xT[b],
                "wqkB": np.ascontiguousarray(wqkB),
                "wvT": np.ascontiguousarray(wvT),
                "woT": np.ascontiguousarray(woT),
                "cosT": cosT,
                "sinT": sinT_signed,
            }
        )
    return in_maps


def assemble_output(results):
    """Sum the 4 o-proj fp16 partials per batch (in fp32) and transpose."""
    out = np.zeros((B, N, HID), dtype=np.float32)
    for c, res in enumerate(results):
        b = c // 4
        out[b] += res["outT"].T.astype(np.float32)
    return out


def kernel(hidden_states, cos, sin, w_qkv, w_o):
    from concourse.bass_utils import run_bass_kernel_spmd

    nc = get_nc()
    in_maps = make_in_maps(hidden_states, cos, sin, w_qkv, w_o)
    res = run_bass_kernel_spmd(nc, in_maps, core_ids=list(range(8)))
    return assemble_output(res.results)


# revision 6
# speedup vs baseline: 1.0161x; 1.0011x over previous
"""Multi-head attention (QKV proj + RoPE + softmax attention + o-proj) on 8
Trainium2 NeuronCores.

Sharding: data-parallel over batch (B=2) x tensor-parallel over heads
(16 heads -> 4 groups of 4). Core c handles batch c//4, heads 4*(c%4)..+4.
qkv_proj is column-parallel, o_proj row-parallel; each core returns a
partial o-proj output (fp16) and the host sums the 4 partials per batch.

All matmuls run in fp16 (full PE speed) with fp32 PSUM accumulation.

Schedule notes (per core):
 - Phase A stripe 0 runs kt-OUTER: all 8 q/k chains accumulate in 8
   dedicated PSUM banks, one matmul per (kt, m), so the PE consumes x/w
   tiles in DMA-arrival order. The DMA stream interleaves wqk/x0 by
   kt-pair, so the first matmul fires ~1.5us in and the PE never starves
   -> HAM warms at ~5us instead of ~24us.
 - RoPE drains are split: bank-freeing reads (rot copies on ACT, t2 on
   DVE) are emitted at chain end; the t1/add finishes are deferred into
   the v-chain window so stripe-0 banks free at ACT pace.
 - q,k are produced as [dh, tok]; scores are computed transposed
   (S^T = k^T q) so softmax's k-reduction lands on matmul-friendly axes.
 - softmax: exp on the scalar engine (scale folded in); the denominator is
   a pairwise fp16 tree-sum of the exp tiles on DVE plus one all-ones
   matmul partition-reduce; normalize via fast approximate reciprocal.
 - The PE is kept at full duty through phase B by interleaving one extra
   matmul per kt slot: o-proj matmuls of the previous token stripe (and,
   for the first stripe, the deferred stripe-3 q-projection chains). The
   last kt-pair's PV matmuls are emitted after the filler flush so the
   trailing exp latency is hidden at stripe boundaries.
 - Output is fp16 (host upcasts and sums partials), written with paired
   (2x128-row) DMA dispatches; the last stripe's softmax denominators are
   emitted per-head so the tail o-proj burst starts as early as possible.
"""

import sys

if "/opt/trn_rl_repo" not in sys.path:
    sys.path.insert(0, "/opt/trn_rl_repo")

import numpy as np

import concourse.bass as bass
import concourse.mybir as mybir
import concourse.tile as tile
from concourse.tile import add_dep_helper
from concourse import bacc

B, N, HID, H = 2, 2048, 2048, 16
DH = 128
HPC = 4  # heads per core
P = 128
F16 = mybir.dt.float16
F32 = mybir.dt.float32
SCALE = 1.0 / float(np.sqrt(DH))

_NC_CACHE = [None]


def build_nc():
    nc = bacc.Bacc(None, target_bir_lowering=False)

    KT = HID // P  # 16 contraction tiles over hid
    NT = N // P  # 16 token tiles
    NS = N // 512  # 4 token stripes
    MQK = 2 * HPC  # 8 output dim-tiles for q+k

    xT = nc.dram_tensor("xT", [HID, N], F16, kind="ExternalInput")
    # wqk blocked on host, kt-major: wqkB[p, kt, m, c] = w[m*P+c, kt*P+p]
    wqkB = nc.dram_tensor("wqkB", [P, KT * MQK * P], F16, kind="ExternalInput")
    wvT = nc.dram_tensor("wvT", [HID, HPC * DH], F16, kind="ExternalInput")
    woT = nc.dram_tensor("woT", [HPC * DH, HID], F16, kind="ExternalInput")
    cosT = nc.dram_tensor("cosT", [DH, N], F16, kind="ExternalInput")
    sinT = nc.dram_tensor("sinT", [DH, N], F16, kind="ExternalInput")
    outT = nc.dram_tensor("outT", [HID, N], F16, kind="ExternalOutput")

    mult = mybir.AluOpType.mult
    add = mybir.AluOpType.add
    Exp = mybir.ActivationFunctionType.Exp

    wqkB4 = wqkB[:].rearrange("p (kt m c) -> p kt m c", kt=KT, m=MQK)

    def x_src(s, kt0, kt1):
        # xT rows for kt tiles [kt0, kt1), stripe s columns, as [p, kt, n]
        return xT[kt0 * P : kt1 * P, s * 512 : (s + 1) * 512].rearrange(
            "(kt p) n -> p kt n", p=P
        )

    with tile.TileContext(nc) as tc:
        with (
            tc.tile_pool(name="const", bufs=1) as const,
            tc.tile_pool(name="persist", bufs=1) as persist,
        ):
            warm_src = const.tile([P, 512], F16, tag="warmsrc")
            nc.vector.memset(warm_src[:], 0.0)
            ones_sb = const.tile([P, P], F16, tag="ones")

            wqk_sb = const.tile([P, KT, MQK, P], F16, tag="wqk")
            cos_sb = const.tile([P, N], F16, tag="cos")
            sin_sb = const.tile([P, N], F16, tag="sin")
            nc.vector.memset(ones_sb[:], 1.0)

            # persistent intermediates
            qk_tiles = [
                persist.tile([P, N], F16, tag=f"qk{m}", name=f"qk{m}")
                for m in range(MQK)
            ]
            v_sb = persist.tile([P, NT, HPC * DH], F16, tag="v")

            # ---------------- Phase A: QKV + RoPE ----------------
            # (stripe 3's q chains are deferred into phase B's filler slots)
            with (
                tc.tile_pool(name="aphase", bufs=2) as aphase,
                tc.tile_pool(name="psumA", bufs=1, space="PSUM") as psumA,
            ):
                # x stripes 0-2 die with this pool; stripe 3 persists for the
                # phase-B q3 filler chains.
                x_stripes = [
                    (persist if s == NS - 1 else aphase).tile(
                        [P, KT, 512], F16, tag=f"xs{s}", name=f"xs{s}", bufs=1
                    )
                    for s in range(NS)
                ]
                wv_sb = aphase.tile([P, KT, HPC * DH], F16, tag="wv", bufs=1)
                # Startup loads: ONE ordered stream on the sync queue — ring
                # back-pressure makes transfers follow dispatch order, and
                # the order below is phase-A consumption order.  Stripe 0 is
                # interleaved by kt-PAIR (wqk then x) to match the kt-outer
                # matmul order of stripe 0.
                # first two kt slots at single-kt granularity so the very
                # first chain matmuls can fire as early as possible
                for kt0 in range(2):
                    nc.sync.dma_start(
                        wqk_sb[:, kt0 : kt0 + 1, :, :],
                        wqkB4[:, kt0 : kt0 + 1, :, :],
                    )
                    nc.sync.dma_start(
                        x_stripes[0][:, kt0 : kt0 + 1, :],
                        x_src(0, kt0, kt0 + 1),
                    )
                for kt0 in range(2, KT, 2):
                    nc.sync.dma_start(
                        wqk_sb[:, kt0 : kt0 + 2, :, :],
                        wqkB4[:, kt0 : kt0 + 2, :, :],
                    )
                    nc.sync.dma_start(
                        x_stripes[0][:, kt0 : kt0 + 2, :],
                        x_src(0, kt0, kt0 + 2),
                    )
                nc.sync.dma_start(cos_sb[:, 0:512], cosT[:, 0:512])
                nc.sync.dma_start(sin_sb[:, 0:512], sinT[:, 0:512])
                nc.sync.dma_start(
                    wv_sb[:], wvT[:].rearrange("(kt p) m -> p kt m", p=P)
                )
                nc.sync.dma_start(x_stripes[1][:], x_src(1, 0, KT))
                nc.sync.dma_start(cos_sb[:, 512:N], cosT[:, 512:N])
                nc.sync.dma_start(sin_sb[:, 512:N], sinT[:, 512:N])
                nc.sync.dma_start(x_stripes[2][:], x_src(2, 0, KT))
                nc.sync.dma_start(x_stripes[3][:], x_src(3, 0, KT))

                # 8 dedicated PSUM banks for the whole of phase A:
                # qk chain m -> bank m; v chain tt -> bank tt (reused).
                ps_m = [
                    psumA.tile([P, 512], F32, tag=f"pa{m}", name=f"pa{m}")
                    for m in range(MQK)
                ]

                # HAM warmup: short matmuls while the first DMAs land.
                for _ in range(8):
                    nc.tensor.matmul(
                        ps_m[0][:, 0:256], warm_src[:, 0:P], warm_src[:, 0:256],
                        start=True, stop=True,
                    )

                def emit_rope_drain(s, m, ps):
                    # bank-freeing reads of ps only (rot on ACT, t2 on DVE);
                    # rot/t2 are kept in fp16 so stripe 0 can hold all 8.
                    sl = slice(s * 512, (s + 1) * 512)
                    rot = aphase.tile([P, 512], F16, tag="rot", bufs=9,
                                      name="rot")
                    nc.scalar.copy(rot[0:64, :], ps[64:128, :])
                    nc.scalar.copy(rot[64:128, :], ps[0:64, :])
                    t2 = aphase.tile([P, 512], F16, tag="t2", bufs=9, name="t2")
                    nc.vector.tensor_tensor(t2[:], ps[:], cos_sb[:, sl], mult)
                    return rot, t2

                def emit_rope_finish(s, m, rot, t2):
                    sl = slice(s * 512, (s + 1) * 512)
                    t1 = aphase.tile([P, 512], F16, tag="t1", bufs=2, name="t1")
                    nc.vector.tensor_tensor(t1[:], rot[:], sin_sb[:, sl], mult)
                    nc.vector.tensor_tensor(qk_tiles[m][:, sl], t1[:], t2[:], add)

                def emit_qk_chain(s, m):
                    # m-outer chain for stripes 1+ (data already resident)
                    ps = ps_m[m]
                    for kt in range(KT):
                        nc.tensor.matmul(
                            ps[:],
                            wqk_sb[:, kt, m, :],
                            x_stripes[s][:, kt, :],
                            start=(kt == 0),
                            stop=(kt == KT - 1),
                        )
                    rot, t2 = emit_rope_drain(s, m, ps)
                    emit_rope_finish(s, m, rot, t2)

                def emit_v_chain(s, tt, fast_drain=False):
                    tok = s * 4 + tt
                    psv = ps_m[tt]
                    for kt in range(KT):
                        nc.tensor.matmul(
                            psv[:],
                            x_stripes[s][:, kt, tt * P : (tt + 1) * P],
                            wv_sb[:, kt, :],
                            start=(kt == 0),
                            stop=(kt == KT - 1),
                        )
                    if fast_drain:
                        # DVE is idle during the last stripe's v chains; the
                        # activation queue would serialize these behind the
                        # RoPE rot-copy backlog right before phase B needs
                        # the psum banks back.
                        nc.vector.tensor_copy(v_sb[:, tok, :], psv[:])
                    else:
                        nc.scalar.copy(v_sb[:, tok, :], psv[:])

                # ---- stripe 0: kt-outer so the PE follows the DMA stream
                pend_rope = []
                for kt in range(KT):
                    for m in range(MQK):
                        nc.tensor.matmul(
                            ps_m[m][:],
                            wqk_sb[:, kt, m, :],
                            x_stripes[0][:, kt, :],
                            start=(kt == 0),
                            stop=(kt == KT - 1),
                        )
                        if kt == KT - 1:
                            pend_rope.append(
                                (m, emit_rope_drain(0, m, ps_m[m]))
                            )
                for tt in range(4):
                    # fast_drain: the scalar queue is backlogged with the 16
                    # stripe-0 rot copies; DVE drains free the psum bank in
                    # time for stripe 1's chains.
                    emit_v_chain(0, tt, fast_drain=True)
                    # rope finishes ride the idle DVE during the v window
                    for _ in range(2):
                        if pend_rope:
                            m, (rot, t2) = pend_rope.pop(0)
                            emit_rope_finish(0, m, rot, t2)

                # ---- stripes 1-3: m-outer chains
                for s in range(1, NS):
                    last = s == NS - 1
                    m_list = range(MQK) if not last else range(HPC, MQK)
                    for m in m_list:
                        emit_qk_chain(s, m)
                    for tt in range(4):
                        emit_v_chain(s, tt, fast_drain=last)

            # ---------------- Phases B (attention) + C (o-proj) ----------
            with tc.tile_pool(name="late", bufs=3) as late:
                wo_sb = late.tile([P, HPC, HID], F16, tag="wo", bufs=1)
                nc.sync.dma_start(
                    wo_sb[:], woT[:].rearrange("(ht p) o -> p ht o", p=P)
                )
                attn_tiles = [
                    late.tile([P, N], F16, tag=f"attn{h}", name=f"attn{h}", bufs=1)
                    for h in range(HPC)
                ]

                with tc.tile_pool(name="psumB", bufs=2, space="PSUM") as psumB:
                    pending = []  # deferred denominator/normalize closure args

                    def emit_tree(accs):
                        # pairwise fp16 tree-sum of the 8 exp accumulators;
                        # emitted at head end so the DVE work is done before
                        # the denominator matmul lands on the (in-order) PE
                        # queue at the pop point.
                        stride = 1
                        while stride < 8:
                            for i in range(0, 8, 2 * stride):
                                nc.vector.tensor_tensor(
                                    accs[i][:], accs[i][:], accs[i + stride][:], add
                                )
                            stride *= 2

                    def emit_denorm(accs, outp, h, qsl):
                        den = psumB.tile([P, 512], F32, tag="cps", bufs=2, name="den")
                        nc.tensor.matmul(
                            den[:], ones_sb[:], accs[0][:], start=True, stop=True
                        )
                        rec = late.tile([P, 512], F32, tag="rec", bufs=2)
                        nc.vector.reciprocal_approx_fast(rec[:], den[:])
                        nc.vector.tensor_tensor(
                            attn_tiles[h][:, qsl], outp[:], rec[:], mult
                        )

                    def make_c_fillers(ts, tag="cps", co=None):
                        # one closure per o-proj matmul for token stripe ts,
                        # spread through the next stripe's kt slots to keep
                        # the PE at full duty. Output DMAs are paired over
                        # two consecutive ho blocks (one dispatch per pair).
                        tsl = slice(ts * 512, (ts + 1) * 512)
                        state = {}
                        if co is None:
                            co = {}
                        fillers = []

                        def mk(ho, hi, tag="cps"):
                            def emit():
                                if hi == 0:
                                    state[ho] = psumB.tile(
                                        [P, 512], F32, tag=tag, bufs=2, name="cps"
                                    )
                                nc.tensor.matmul(
                                    state[ho][:],
                                    wo_sb[:, hi, ho * P : (ho + 1) * P],
                                    attn_tiles[hi][:, tsl],
                                    start=(hi == 0),
                                    stop=(hi == HPC - 1),
                                )
                                if hi == HPC - 1:
                                    if ho % 2 == 0:
                                        co[ho // 2] = late.tile(
                                            [P, 2, 512], F16, tag="co", bufs=3,
                                            name="co",
                                        )
                                    ob = co[ho // 2]
                                    nc.vector.tensor_copy(
                                        ob[:, ho % 2, :], state[ho][:]
                                    )
                                    if ho % 2 == 1:
                                        nc.sync.dma_start(
                                            outT[
                                                (ho - 1) * P : (ho + 1) * P, tsl
                                            ].rearrange("(i p) n -> p i n", p=P),
                                            ob[:],
                                        )
                            return emit

                        for ho in range(HID // P):
                            for hi in range(HPC):
                                fillers.append(mk(ho, hi, tag=tag))
                        return fillers

                    def make_q3_fillers():
                        # deferred stripe-3 q-projection chains (m=0..3),
                        # one matmul per filler slot + RoPE at chain end.
                        s3 = NS - 1
                        sl3 = slice(s3 * 512, (s3 + 1) * 512)
                        x3 = x_stripes[s3]
                        state = {}
                        fillers = []

                        def mk(m, kt):
                            def emit():
                                if kt == 0:
                                    state[m] = psumB.tile(
                                        [P, 512], F32, tag="cps", bufs=2, name="q3ps"
                                    )
                                nc.tensor.matmul(
                                    state[m][:],
                                    wqk_sb[:, kt, m, :],
                                    x3[:, kt, :],
                                    start=(kt == 0),
                                    stop=(kt == KT - 1),
                                )
                                if kt == KT - 1:
                                    ps = state[m]
                                    rot = late.tile([P, 512], F32, tag="rot3", bufs=2)
                                    nc.vector.tensor_copy(rot[0:64, :], ps[64:128, :])
                                    nc.vector.tensor_copy(rot[64:128, :], ps[0:64, :])
                                    t1 = late.tile([P, 512], F32, tag="t13", bufs=2)
                                    nc.vector.tensor_tensor(
                                        t1[:], rot[:], sin_sb[:, sl3], mult
                                    )
                                    t2 = late.tile([P, 512], F32, tag="t23", bufs=2)
                                    nc.vector.tensor_tensor(
                                        t2[:], ps[:], cos_sb[:, sl3], mult
                                    )
                                    nc.vector.tensor_tensor(
                                        qk_tiles[m][:, sl3], t1[:], t2[:], add
                                    )
                            return emit

                        for m in range(HPC):
                            for kt in range(KT):
                                fillers.append(mk(m, kt))
                        return fillers

                    def emit_pv_acc(pt, outp, accs, h, kt):
                        pvs = []
                        for half in range(2):
                            k2 = kt - 1 + half
                            pv = nc.tensor.matmul(
                                outp[:],
                                v_sb[:, k2, h * DH : (h + 1) * DH],
                                pt[:, half * 512 : (half + 1) * 512],
                                start=(k2 == 0),
                                stop=(k2 == NT - 1),
                            )
                            pvs.append(pv)
                        acc = late.tile(
                            [P, 512], F16, tag="acc", bufs=10, name="acc"
                        )
                        ai = nc.vector.tensor_tensor(
                            acc[:], pt[:, 0:512], pt[:, 512:1024], add
                        )
                        for pv in pvs:
                            add_dep_helper(ai.ins, pv.ins,
                                           reason="pt SBUF contention")
                        accs.append(acc)

                    # Tail o-proj chain lists are created up-front so the
                    # first chains' hi=0..2 matmuls can be appended to the
                    # last stripe's filler list (they are independent of the
                    # last head's softmax and cover its denominator latency).
                    tail_f = []
                    tail_co = {}
                    for tag in ("cps", "po", "st"):
                        tail_f.append(make_c_fillers(NS - 1, tag=tag, co=tail_co))
                    tassign = {0: 1, 1: 2, 2: 2, 3: 0, 4: 0}
                    for ho in range(5, HID // P):
                        tassign[ho] = [1, 2, 0][(ho - 5) % 3]

                    def TF(ho, hi):
                        return tail_f[tassign[ho]][ho * HPC + hi]

                    tail_den = []

                    for qs in range(NS):
                        last_stripe = qs == NS - 1
                        qsl = slice(qs * 512, (qs + 1) * 512)
                        fillers = (
                            make_c_fillers(qs - 1) if qs > 0 else make_q3_fillers()
                        )
                        if last_stripe:
                            # 3 tail chains' hi=0..2 ride the filler slots at
                            # the stripe end (attn[3]-independent PE work)
                            for hi in range(3):
                                for ho in range(3):
                                    fillers.append(TF(ho, hi))
                        fi = 0
                        for h in range(HPC):
                            last_head = h == HPC - 1
                            kT_tile = qk_tiles[HPC + h]
                            qT_tile = qk_tiles[h]
                            outp = psumB.tile([P, 512], F32, tag="po", bufs=2)
                            accs = []
                            st2 = None
                            deferred_pv = None
                            for kt in range(NT):
                                if kt % 2 == 0:
                                    st2 = psumB.tile(
                                        [P, 1024], F32, tag="st", bufs=2, name="st2"
                                    )
                                nc.tensor.matmul(
                                    st2[:, (kt % 2) * 512 : (kt % 2 + 1) * 512],
                                    kT_tile[:, kt * P : (kt + 1) * P],
                                    qT_tile[:, qsl],
                                    start=True,
                                    stop=True,
                                )
                                pt = None
                                if kt % 2 == 1:
                                    # one 1024-wide exp per ST pair (halves the
                                    # scalar-engine instruction/sem count)
                                    pt = late.tile([P, 1024], F16, tag="pt", bufs=3)
                                    nc.scalar.activation(
                                        pt[:], st2[:], Exp, scale=SCALE
                                    )
                                if kt == 1 and pending:
                                    emit_denorm(*pending.pop())
                                # reserve 5 fillers for the deferred-PV flush
                                # at the stripe end (covers the last exp's
                                # latency on the in-order PE queue)
                                if (h > 0 or kt >= 2) and fi < len(fillers) - 5:
                                    fillers[fi]()
                                    fi += 1
                                if pt is not None:
                                    if last_head and kt == NT - 1:
                                        # defer the trailing PVs past the filler
                                        # flush so the exp latency is hidden
                                        deferred_pv = pt
                                    else:
                                        emit_pv_acc(pt, outp, accs, h, kt)
                            if deferred_pv is not None:
                                while fi < len(fillers):
                                    fillers[fi]()
                                    fi += 1
                                emit_pv_acc(deferred_pv, outp, accs, h, NT - 1)
                            emit_tree(accs)
                            if last_stripe:
                                if last_head:
                                    # denominator deferred into the tail so
                                    # its DVE tree latency is covered by
                                    # chain-3's hi0-2 matmuls
                                    tail_den.append((accs, outp, h, qsl))
                                else:
                                    # immediate denorm so the tail o-proj
                                    # burst can start as early as possible
                                    emit_denorm(accs, outp, h, qsl)
                            else:
                                pending.append((accs, outp, h, qsl))
                        while fi < len(fillers):
                            fillers[fi]()
                            fi += 1
                    # ---- tail: remaining o-proj for the final stripe.
                    # Chains 0-2 already have hi0-2 in flight (filler list).
                    # Order keeps the in-order PE queue busy across the last
                    # head's tree -> den -> recip -> normalize chain.
                    for hi in range(3):
                        TF(3, hi)()
                    emit_denorm(*tail_den[0])
                    for ho in range(4):
                        TF(ho, 3)()
                    for hi in range(4):
                        TF(4, hi)()
                    for ho in range(5, HID // P):
                        for hi in range(HPC):
                            TF(ho, hi)()

    nc.finalize()
    return nc


def get_nc():
    if _NC_CACHE[0] is None:
        _NC_CACHE[0] = build_nc()
    return _NC_CACHE[0]


def make_in_maps(hidden_states, cos, sin, w_qkv, w_o):
    """Build the 8 per-core input maps (host-side shard + transpose + cast)."""
    hidden_states = np.asarray(hidden_states, dtype=np.float32)
    cos = np.asarray(cos, dtype=np.float32)
    sin = np.asarray(sin, dtype=np.float32)
    w_qkv = np.asarray(w_qkv, dtype=np.float32)
    w_o = np.asarray(w_o, dtype=np.float32)

    KT = HID // P
    MQK = 2 * HPC

    cosT = np.ascontiguousarray(cos.T).astype(np.float16)  # [DH, N]
    sinT_signed = np.ascontiguousarray(
        np.concatenate([-sin.T[: DH // 2], sin.T[DH // 2 :]], axis=0)
    ).astype(np.float16)

    xT = [
        np.ascontiguousarray(hidden_states[b].T).astype(np.float16)
        for b in range(B)
    ]

    in_maps = []
    for c in range(8):
        b, g = divmod(c, 4)
        qrows = slice(g * HPC * DH, (g + 1) * HPC * DH)
        krows = slice(HID + g * HPC * DH, HID + (g + 1) * HPC * DH)
        vrows = slice(2 * HID + g * HPC * DH, 2 * HID + (g + 1) * HPC * DH)
        # blocked wqk, kt-major: wqkB[p, kt, m, c] = wqk_rows[m*P+c, kt*P+p]
        wqk_rows = np.concatenate([w_qkv[qrows], w_qkv[krows]], axis=0)
        wqkB = (
            wqk_rows.reshape(MQK, P, KT, P)
            .transpose(3, 2, 0, 1)
            .reshape(P, KT * MQK * P)
            .astype(np.float16)
        )
        wvT = w_qkv[vrows].T.astype(np.float16)
        woT = w_o[:, g * HPC * DH : (g + 1) * HPC * DH].T.astype(np.float16)
        in_maps.append(
            {
                "xT": xT[b],
                "wqkB": np.ascontiguousarray(wqkB),
                "wvT": np.ascontiguousarray(wvT),
                "woT": np.ascontiguousarray(woT),
                "cosT": cosT,
                "sinT": sinT_signed,
            }
        )
    return in_maps


def assemble_output(results):
    """Sum the 4 o-proj fp16 partials per batch (in fp32) and transpose."""
    out = np.zeros((B, N, HID), dtype=np.float32)
    for c, res in enumerate(results):
        b = c // 4
        out[b] += res["outT"].T.astype(np.float32)
    return out


def kernel(hidden_states, cos, sin, w_qkv, w_o):
    from concourse.bass_utils import run_bass_kernel_spmd

    nc = get_nc()
    in_maps = make_in_maps(hidden_states, cos, sin, w_qkv, w_o)
    res = run_bass_kernel_spmd(nc, in_maps, core_ids=list(range(8)))
    return assemble_output(res.results)
